# revision 18
# baseline (speedup 1.0000x reference)
"""Two-layer Elman RNN (B=64, S=512, EMB=512, HID=1024) on 8 TRN2 NeuronCores.

Layer-pipelined pairs: pair p = (core p, core p+4) handles batch quarter p
(16 rows). Core p runs the layer-1 scan and streams pre2 = h1 @ Wi2 + b2
chunks (32 steps each) to core p+4 via pair-wise AllGather; core p+4 runs the
layer-2 scan one chunk behind and produces the output quarter. All 8 cores
execute an identical SPMD program — roles differ only in input data (scan
weights Wh1 vs Wh2, and blend scalars alpha/beta selecting local-pre1 vs
received-pre2 as the scan input).

Activation layout: feature-major, columns ordered (t, k, b); the per-chunk
h archive is split into two tiles (k<4 / k>=4) and each scan step runs as
two PSUM passes — pass E accumulates all k<4 weight tiles (which only read
the previous step's early tanh half) into one bank, pass O accumulates k>=4
into two further banks so the tanh of output half A lands while the PE is
still streaming half B. This keeps the TensorEngine weight-load stream (the
real-HW bottleneck: 64 fp16 [128,128] LDWEIGHTS per step, FWL 2 cols/cycle
= ~3.4us/step) free of cross-engine stalls. Matmuls in fp16 (fp32 PSUM
accumulate), tanh/sigmoid in fp32. The zero fixed point of h = tanh(W h + 0)
makes the warm-up chunk of the layer-2 cores (fed zeros) end exactly in the
correct initial state h = 0.
"""

import os
from contextlib import ExitStack

import numpy as np

import concourse.bass as bass
import concourse.bacc as bacc
import concourse.mybir as mybir
import concourse.tile as tile
from concourse.bass import IndirectOffsetOnAxis
from concourse.bass_utils import run_bass_kernel_spmd
from concourse.masks import make_identity

P = 128
VOCAB, EMB, HID = 50257, 512, 1024
B, S = 64, 512
NCORES = 8
NPAIR = 4
BL = B // NPAIR           # batch rows per pair = 16
KE = EMB // P             # 4
KH = HID // P             # 8
M = HID // P              # 8
CS = 32                   # scan steps per chunk
NCH = S // CS             # 16 chunks
CCOL = BL * CS            # 512 token-columns per chunk
W = M * BL                # 128 columns per step block (m, b)
CW = CS * W               # 4096 columns per chunk in combined layout

CDT = mybir.dt.float16
NP_CDT = np.float16
F32 = mybir.dt.float32
I32 = mybir.dt.int32

COLT = int(os.environ.get("KERNEL_COLT", "1"))   # scan col-tiling factor
_BUILT = {}
REPLICA_GROUPS = [[p, p + NPAIR] for p in range(NPAIR)]


def _tb(ap):
    """View a [P, CCOL] (t,b)-ordered AP as [P, CS, BL]."""
    return ap.rearrange("p (t b) -> p t b", b=BL)


def _tmb(ap, m):
    """m-th [P, CS, BL] slice of a [P, CW] combined (t,m,b) AP."""
    return ap.rearrange("p (t mb) -> p t mb", mb=W)[:, :, m * BL:(m + 1) * BL]


def build(local_cc=False):
    """local_cc=True replaces the AllGather with an equivalent-volume local
    DMA so the collective-free program can run under TimelineSim."""
    nc = bacc.Bacc("TRN2", target_bir_lowering=False, debug=False, num_devices=NCORES)

    xg = nc.dram_tensor("xg", [S * BL // P, P], I32, kind="ExternalInput").ap()
    emb_d = nc.dram_tensor("emb", [VOCAB, EMB], CDT, kind="ExternalInput").ap()
    wi1_d = nc.dram_tensor("wi1", [EMB, HID], CDT, kind="ExternalInput").ap()
    whs_d = nc.dram_tensor("whs", [HID, HID], CDT, kind="ExternalInput").ap()
    wsend_d = nc.dram_tensor("wsend", [HID, HID], CDT, kind="ExternalInput").ap()
    b1_d = nc.dram_tensor("b1m", [M, P], F32, kind="ExternalInput").ap()
    bs_d = nc.dram_tensor("bsm", [M, P], F32, kind="ExternalInput").ap()
    ab_d = nc.dram_tensor("ab", [2, P], F32, kind="ExternalInput").ap()
    wd_d = nc.dram_tensor("wdk", [P, KH], CDT, kind="ExternalInput").ap()
    bd_d = nc.dram_tensor("bdv", [BL], F32, kind="ExternalInput").ap()
    y_d = nc.dram_tensor("y", [BL], F32, kind="ExternalOutput").ap()

    AF = mybir.ActivationFunctionType
    ALU = mybir.AluOpType

    with tile.TileContext(nc) as tc, ExitStack() as ctx:
        const_pool = ctx.enter_context(tc.tile_pool(name="const", bufs=1))
        wpool = ctx.enter_context(tc.tile_pool(name="weights", bufs=1))
        dpool = ctx.enter_context(tc.tile_pool(name="dram", bufs=1, space="DRAM"))
        cpool = ctx.enter_context(tc.tile_pool(name="ccdram", bufs=2, space="DRAM"))
        gpool = ctx.enter_context(tc.tile_pool(name="gather", bufs=4))
        xpool = ctx.enter_context(tc.tile_pool(name="xet", bufs=2))
        lpool = ctx.enter_context(tc.tile_pool(name="locpre", bufs=2))
        prepool = ctx.enter_context(tc.tile_pool(name="prework", bufs=2))
        rpool = ctx.enter_context(tc.tile_pool(name="recv", bufs=2))
        apool = ctx.enter_context(tc.tile_pool(name="arch", bufs=2))
        spool = ctx.enter_context(tc.tile_pool(name="send", bufs=2))
        bigps = ctx.enter_context(tc.tile_pool(name="bigps", bufs=3, space="PSUM"))
        spsumE = ctx.enter_context(tc.tile_pool(name="spsumE", bufs=2, space="PSUM"))
        spsumO = ctx.enter_context(tc.tile_pool(name="spsumO", bufs=1, space="PSUM"))

        ident = const_pool.tile([P, P], CDT, name="ident")
        make_identity(nc, ident[:])
        b1t = const_pool.tile([P, M], F32, name="b1t")
        nc.sync.dma_start(out=b1t[:], in_=b1_d.rearrange("m p -> p m"))
        bst = const_pool.tile([P, M], F32, name="bst")
        nc.sync.dma_start(out=bst[:], in_=bs_d.rearrange("m p -> p m"))
        abt = const_pool.tile([P, 2], F32, name="abt")
        nc.sync.dma_start(out=abt[:], in_=ab_d.rearrange("a p -> p a"))
        alpha, beta = abt[:, 0:1], abt[:, 1:2]
        wd_sb = const_pool.tile([P, KH], CDT, name="wd_sb")
        nc.sync.dma_start(out=wd_sb[:], in_=wd_d[:])
        bd_sb = const_pool.tile([P, 1], F32, name="bd_sb")
        nc.sync.dma_start(out=bd_sb[0:BL, 0:1], in_=bd_d[:])

        wi_sb = wpool.tile([P, KE * M * P], CDT, name="wi_sb")
        for e in range(KE):
            nc.sync.dma_start(out=wi_sb[:, e * HID:(e + 1) * HID],
                              in_=wi1_d[e * P:(e + 1) * P, :])
        whs_sb = wpool.tile([P, KH * M * P], CDT, name="whs_sb")
        for k in range(KH):
            nc.sync.dma_start(out=whs_sb[:, k * HID:(k + 1) * HID],
                              in_=whs_d[k * P:(k + 1) * P, :])
        wsend_sb = wpool.tile([P, KH * M * P], CDT, name="wsend_sb")
        for k in range(KH):
            nc.sync.dma_start(out=wsend_sb[:, k * HID:(k + 1) * HID],
                              in_=wsend_d[k * P:(k + 1) * P, :])

        # local pre1 staging in DRAM, combined (t,m,b) layout per chunk
        pre1_dram = dpool.tile([P, NCH * CW], CDT, space="DRAM", name="pre1d")

        zrecv = const_pool.tile([P, CW], CDT, name="zrecv")
        nc.vector.memset(zrecv[:], 0.0)

        # ---- Phase A+B: embed + pre1 per chunk, staged to DRAM ----
        for c in range(NCH):
            xeT = [xpool.tile([P, CCOL], CDT, tag=f"xeT{e}", name=f"xeT{c}_{e}")
                   for e in range(KE)]
            for gi in range(CCOL // P):
                g = c * (CCOL // P) + gi
                idx = gpool.tile([P, 1], I32, tag="idx", name=f"idx{g}")
                nc.sync.dma_start(out=idx[:, 0:1], in_=xg[g, :])
                xe_g = gpool.tile([P, EMB], CDT, tag="xe", name=f"xe{g}")
                nc.gpsimd.indirect_dma_start(
                    out=xe_g[:], out_offset=None, in_=emb_d[:],
                    in_offset=IndirectOffsetOnAxis(ap=idx[:, 0:1], axis=0))
                for e in range(KE):
                    pt = bigps.tile([P, P], CDT, tag="ps", name=f"tp{g}_{e}")
                    nc.tensor.transpose(out=pt[:], in_=xe_g[:, e * P:(e + 1) * P],
                                        identity=ident[:])
                    nc.vector.tensor_copy(out=xeT[e][:, gi * P:(gi + 1) * P],
                                          in_=pt[:])
            pc = lpool.tile([P, CW], CDT, tag="pb", name=f"preb{c}")
            for m in range(M):
                ps = bigps.tile([P, CCOL], F32, tag="ps", name=f"ppB{c}_{m}")
                for e in range(KE):
                    nc.tensor.matmul(
                        ps[:], lhsT=wi_sb[:, (e * M + m) * P:(e * M + m + 1) * P],
                        rhs=xeT[e][:, :], start=(e == 0), stop=(e == KE - 1))
                nc.scalar.activation(out=_tmb(pc[:], m), in_=_tb(ps[:]),
                                     func=AF.Identity, bias=b1t[:, m:m + 1])
            nc.sync.dma_start(out=pre1_dram[:, c * CW:(c + 1) * CW], in_=pc[:])

        # ---- Main pipelined loop ----
        arch_prev = None
        recv_prev = zrecv
        cw = P // COLT
        for c in range(NCH + 1):
            lc = min(c, NCH - 1)
            # stream in local pre1 chunk, blend with received chunk
            loc = lpool.tile([P, CW], CDT, tag="loc", name=f"loc{c}")
            nc.sync.dma_start(out=loc[:],
                              in_=pre1_dram[:, lc * CW:(lc + 1) * CW])
            tmp = prepool.tile([P, CW], CDT, tag="tmp", name=f"tmp{c}")
            nc.vector.tensor_scalar_mul(tmp[:], recv_prev[:], beta)
            PRE = prepool.tile([P, CW], CDT, tag="PRE", name=f"PRE{c}")
            nc.vector.scalar_tensor_tensor(
                out=PRE[:], in0=loc[:], scalar=alpha, in1=tmp[:],
                op0=ALU.mult, op1=ALU.add)

            # scan CS steps; archive split into two tiles (k<4 / k>=4) so
            # next-step matmuls depend only on the tanh half they read
            hw = W // 2
            arch = [apool.tile([P, CS * hw], CDT, tag=f"arch{h}",
                               name=f"arch{c}_{h}") for h in range(2)]
            for t in range(CS):
                if c == 0 and t == 0:
                    for h in range(2):
                        nc.scalar.activation(
                            out=arch[h][:, 0:hw],
                            in_=PRE[:, h * hw:(h + 1) * hw], func=AF.Tanh)
                    continue
                rsrc = arch if t > 0 else arch_prev
                rt = t - 1 if t > 0 else CS - 1
                # Two passes on two PSUM banks: pass E accumulates k<4 (reads
                # the previous step's EARLY tanh half), pass O accumulates
                # k>=4 (late half, first needed 32 matmuls into the step).
                # Within each bank the accumulation groups stay strictly
                # sequential (start=True clears the whole bank's has_written).
                psE = spsumE.tile([P, W], F32, tag="spE", name=f"spE{c}_{t}")

                def mm(ps, po, m, k):
                    kh = KH // 2
                    rb = rsrc[k // kh]
                    ko = (k % kh) * BL
                    for j in range(COLT):
                        nc.tensor.matmul(
                            ps[j * cw:(j + 1) * cw, po:po + BL],
                            lhsT=whs_sb[:, (k * M + m) * P + j * cw:
                                        (k * M + m) * P + (j + 1) * cw],
                            rhs=rb[:, rt * hw + ko:rt * hw + ko + BL],
                            start=(k % kh == 0), stop=(k % kh == kh - 1),
                            tile_position=(0, j * cw))

                # pass E: all k<4 contributions (only needs the previous
                # step's early tanh half)
                for m in range(M):
                    for k in range(KH // 2):
                        mm(psE, m * BL, m, k)
                # bias-add of pre can run as soon as pass E is done
                tmpS = []
                for half in range(2):
                    tS = prepool.tile([P, hw], F32, tag=f"tmps{half}",
                                      name=f"tmps{c}_{t}_{half}")
                    nc.vector.tensor_tensor(
                        out=tS[:], in0=psE[:, half * hw:(half + 1) * hw],
                        in1=PRE[:, t * W + half * hw:t * W + (half + 1) * hw],
                        op=ALU.add)
                    tmpS.append(tS)
                # pass O: k>=4, split across two banks so tanh of half A
                # lands while the PE still works on half B
                for half in range(2):
                    psO = spsumO.tile([P, hw], F32, tag=f"spO{half}",
                                      name=f"spO{c}_{t}_{half}")
                    for m in range(half * (M // 2), (half + 1) * (M // 2)):
                        for k in range(KH // 2, KH):
                            mm(psO, (m % (M // 2)) * BL, m, k)
                    nc.vector.tensor_tensor(
                        out=psO[:], in0=psO[:], in1=tmpS[half][:], op=ALU.add)
                    nc.scalar.activation(
                        out=arch[half][:, t * hw:(t + 1) * hw],
                        in_=psO[:], func=AF.Tanh)
            arch_prev = arch

            if c == NCH:
                break

            # chunk matmul: send = arch @ Wsend + bsend, then pair AllGather
            send_db = cpool.tile([P, CW], CDT, space="DRAM", name=f"send_db{c}")
            snd = spool.tile([P, CW], CDT, tag="snd", name=f"snd{c}")
            for m in range(M):
                ps = bigps.tile([P, CCOL], F32, tag="ps", name=f"ppS{c}_{m}")
                for k in range(KH):
                    rv_ = arch[k // (KH // 2)][:].rearrange(
                        "p (t kb) -> p t kb", kb=(KH // 2) * BL)[
                        :, :, (k % (KH // 2)) * BL:(k % (KH // 2) + 1) * BL]
                    nc.tensor.matmul(
                        ps[:], lhsT=wsend_sb[:, (k * M + m) * P:(k * M + m + 1) * P],
                        rhs=rv_, start=(k == 0), stop=(k == KH - 1))
                nc.scalar.activation(out=_tmb(snd[:], m), in_=_tb(ps[:]),
                                     func=AF.Identity, bias=bst[:, m:m + 1])
            nc.sync.dma_start(out=send_db[:], in_=snd[:])

            recv_db = cpool.tile([2 * P, CW], CDT, space="DRAM", name=f"recv_db{c}")
            if local_cc:
                nc.gpsimd.dma_start(out=recv_db[0:P, :], in_=send_db[:])
            else:
                nc.gpsimd.collective_compute(
                    "AllGather", ALU.bypass, ins=[send_db[:]], outs=[recv_db[:]],
                    replica_groups=REPLICA_GROUPS)
            rv = rpool.tile([P, CW], CDT, tag="rv", name=f"rv{c}")
            nc.sync.dma_start(out=rv[:], in_=recv_db[0:P, :])
            recv_prev = rv

        # ---- head ----
        with tc.tile_pool(name="hps", bufs=1, space="PSUM") as hpool:
            hps = hpool.tile([BL, 1], F32, name="hps")
            hw = W // 2
            for k in range(KH):
                last = arch_prev[k // (KH // 2)][:, (CS - 1) * hw:CS * hw]
                ko = (k % (KH // 2)) * BL
                nc.tensor.matmul(
                    hps[:], lhsT=last[:, ko:ko + BL],
                    rhs=wd_sb[:, k:k + 1], start=(k == 0), stop=(k == KH - 1))
            y_sb = const_pool.tile([P, 1], F32, name="y_sb")
            nc.scalar.activation(out=y_sb[0:BL, 0:1], in_=hps[:],
                                 func=AF.Sigmoid, bias=bd_sb[0:BL, 0:1])
            nc.sync.dma_start(out=y_d[:], in_=y_sb[0:BL, 0:1])

    nc.compile()
    return nc


def _prep_maps(x, emb, Wi1, Wh1, b1, Wi2, Wh2, b2, Wd, bd):
    f = NP_CDT
    x = np.asarray(x, np.int32)
    shared = {
        "emb": np.ascontiguousarray(np.asarray(emb, f)),
        "wi1": np.ascontiguousarray(np.asarray(Wi1, f)),
        "wsend": np.ascontiguousarray(np.asarray(Wi2, f)),
        "b1m": np.ascontiguousarray(np.asarray(b1, np.float32).reshape(M, P)),
        "bsm": np.ascontiguousarray(np.asarray(b2, np.float32).reshape(M, P)),
        "wdk": np.ascontiguousarray(np.asarray(Wd, f).reshape(KH, P).T),
        "bdv": np.ascontiguousarray(np.broadcast_to(
            np.asarray(bd, np.float32), (BL,))),
    }
    wh1 = np.ascontiguousarray(np.asarray(Wh1, f))
    wh2 = np.ascontiguousarray(np.asarray(Wh2, f))
    ab_a = np.stack([np.ones(P, np.float32), np.zeros(P, np.float32)])
    ab_b = np.stack([np.zeros(P, np.float32), np.ones(P, np.float32)])
    in_maps = []
    for c in range(NCORES):
        p = c % NPAIR
        xs = x[p * BL:(p + 1) * BL, :]                    # [16, 512]
        xgrp = np.ascontiguousarray(xs.T).reshape(-1, P)  # (t, b) order
        role_a = c < NPAIR
        in_maps.append({
            **shared,
            "xg": xgrp,
            "whs": wh1 if role_a else wh2,
            "ab": ab_a if role_a else ab_b,
        })
    return in_maps


def kernel(x, emb, Wi1, Wh1, b1, Wi2, Wh2, b2, Wd, bd):
    if "nc" not in _BUILT:
        _BUILT["nc"] = build()
    nc = _BUILT["nc"]
    in_maps = _prep_maps(x, emb, Wi1, Wh1, b1, Wi2, Wh2, b2, Wd, bd)
    res = run_bass_kernel_spmd(nc, in_maps, list(range(NCORES)))
    kernel.last_result = res
    y = np.concatenate([np.asarray(res.results[NPAIR + p]["y"], np.float32)
                        for p in range(NPAIR)])
    return y
